# revision 21
# baseline (speedup 1.0000x reference)
import sys
if "/opt/trn_rl_repo" not in sys.path:
    sys.path.insert(0, "/opt/trn_rl_repo")
from contextlib import ExitStack
import numpy as np
import concourse.bass as bass
import concourse.bacc as bacc
import concourse.tile as tile
import concourse.mybir as mybir

B, N, D, H, R = 4, 2048, 256, 8, 64
DH, K_SP = 32, 32
NCORES = 8
NHALF = N // 2          # rows per core
NBLK = NHALF // 128     # 8 query blocks per core
C_SCALE = float(1.0 / np.sqrt(np.float32(DH)))
F32 = mybir.dt.float32
F16 = mybir.dt.float16
BF16 = mybir.dt.bfloat16
AX = mybir.AxisListType.X
OP = mybir.AluOpType
ACT = mybir.ActivationFunctionType

# packed-weight layout: flat f32, rows of 1024 (all [*,64]/[64,*] = 16 rows)
OFF_VNP, OFF_UQ, OFF_UK, OFF_UV = 0, 16384, 32768, 49152
OFF_VQ, OFF_VK, OFF_VV = 65536, 81920, 98304
OFF_UO, OFF_MR = 114688, 131072
OFF_BIAS = 147456       # bnp | betaf | gamma | betaBN, 256 each
WROWS = 145
WTOT = WROWS * 1024     # 148480

_cached = {}
LAST = None


def _jax_y(x, U):
    import jax
    cpu = jax.devices("cpu")[0]
    xj = jax.device_put(x, cpu)
    Uj = jax.device_put(U, cpu)
    y = (xj @ Uj).reshape(B * N, R)
    return y


def _build_program():
    nc = bacc.Bacc("TRN2", target_bir_lowering=False, debug=False,
                   num_devices=NCORES)
    io = {}
    # own query-half's y = x @ U_np, transposed: [R, NHALF]
    io["yT"] = nc.dram_tensor("yT", [R, NHALF], F32, kind="ExternalInput")
    # packed weights: real data on core 0, zeros elsewhere (AllReduce-bcast)
    io["Wall"] = nc.dram_tensor("Wall", [WROWS, 1024], F32,
                                kind="ExternalInput")
    outZ = nc.dram_tensor("outZ", [NHALF, R], F16, kind="ExternalOutput")
    outC = nc.dram_tensor("outC", [2, D], F32, kind="ExternalOutput")

    with tile.TileContext(nc) as tc, ExitStack() as ctx:
        const = ctx.enter_context(tc.tile_pool(name="const", bufs=1))
        dpool = ctx.enter_context(tc.tile_pool(name="dram", bufs=1,
                                               space="DRAM"))
        # --- collectives: gather pair's y halves, broadcast weights --------
        yb_ = dpool.tile([R, NHALF], F32, name="yb")
        wb_ = dpool.tile([WROWS, 1024], F32, name="wb")
        yg = dpool.tile([2 * R, NHALF], F32, name="yg")
        wg = dpool.tile([WROWS, 1024], F32, name="wg")
        nc.sync.dma_start(yb_[:, :], io["yT"][:, :])
        nc.sync.dma_start(wb_[:, :], io["Wall"][:, :])
        nc.gpsimd.collective_compute(
            "AllGather", OP.bypass,
            replica_groups=[[0, 1], [2, 3], [4, 5], [6, 7]],
            ins=[yb_[:, :].opt()], outs=[yg[:, :].opt()])
        nc.gpsimd.collective_compute(
            "AllReduce", OP.add,
            replica_groups=[list(range(NCORES))],
            ins=[wb_[:, :].opt()], outs=[wg[:, :].opt()])

        def wsl(off, nelem, cols):
            r0 = off // 1024
            nr = nelem // 1024
            return wg[r0:r0 + nr, :].rearrange("a (b c) -> (a b) c", c=cols)

        stgA_cm = tc.tile_pool(name="stgA", bufs=1)
        stgA = stgA_cm.__enter__()

        # fp16 hi/lo split helper: src f32 -> (hi, lo) fp16 via scratch f32
        def mksplit(pool, base, p, f, scope):
            th = pool.tile([p, f], F16, name=f"{base}h", tag=f"{base}h")
            tl = pool.tile([p, f], F16, name=f"{base}l", tag=f"{base}l")
            return th, tl

        # splits represent SC*src (SC=2^12, exact) so the lo residuals stay
        # out of the fp16 denormal-flush zone; consumers rescale by SC^-2
        SC = 8192.0
        ISC2 = float(2.0 ** -26)

        def dosplit(th, tl, src, tmp, scale=SC):
            p, f = th.shape
            nc.scalar.activation(th[:], src, ACT.Copy, scale=scale)
            nc.scalar.activation(tmp[0:p, 0:f], th[:], ACT.Copy)
            nc.vector.scalar_tensor_tensor(
                out=tl[:], in0=src, scalar=scale, in1=tmp[0:p, 0:f],
                op0=OP.mult, op1=OP.subtract)

        tmpS = stgA.tile([128, N], F32, name="tmpS", tag="tmpS")

        # stage-A-scoped f32 tensors
        yf = stgA.tile([64, N], F32, name="yf", tag="yf")
        yq = stgA.tile([64, NHALF], F32, name="yq", tag="yq")
        hT = [stgA.tile([128, N], F32, name=f"hT{i}", tag=f"hT{i}") for i in range(2)]
        hq = [stgA.tile([128, NHALF], F32, name=f"hq{i}", tag=f"hq{i}") for i in range(2)]
        aQ = stgA.tile([64, NHALF], F32, name="aQ", tag="aQ")
        aK = stgA.tile([64, N], F32, name="aK", tag="aK")
        aV = stgA.tile([64, N], F32, name="aV", tag="aV")
        # fp16 splits (stage-A scoped)
        yf_s = mksplit(stgA, "yf", 64, N, stgA)
        yq_s = mksplit(stgA, "yq", 64, NHALF, stgA)
        hT_s = [mksplit(stgA, f"hT{i}", 128, N, stgA) for i in range(2)]
        hq_s = [mksplit(stgA, f"hq{i}", 128, NHALF, stgA) for i in range(2)]
        aQ_s = mksplit(stgA, "aQ", 64, NHALF, stgA)
        aK_s = mksplit(stgA, "aK", 64, N, stgA)
        w_vnp_s = mksplit(stgA, "wvnp", 64, D, stgA)
        w_uq_s = [mksplit(stgA, f"wuq{i}", 128, R, stgA) for i in range(2)]
        w_uk_s = [mksplit(stgA, f"wuk{i}", 128, R, stgA) for i in range(2)]
        w_vq_s = mksplit(stgA, "wvq", 64, D, stgA)
        w_vk_s = mksplit(stgA, "wvk", 64, D, stgA)

        # persistent
        qT_s = [mksplit(const, f"qT{i}", 64, NHALF, const) for i in range(4)]
        kT_s = [mksplit(const, f"kT{i}", 64, N, const) for i in range(4)]
        vv = const.tile([128, 16 * D], F16, name="vv", tag="vv")
        OT = [const.tile([128, NHALF], F32, name=f"OT{i}", tag=f"OT{i}") for i in range(2)]
        w_vnp = stgA.tile([64, D], F32, name="wvnp", tag="wvnp")
        w_uq = [stgA.tile([128, R], F32, name=f"wuq{i}", tag=f"wuq{i}") for i in range(2)]
        w_uk = [stgA.tile([128, R], F32, name=f"wuk{i}", tag=f"wuk{i}") for i in range(2)]
        w_uv = [stgA.tile([128, R], F32, name=f"wuv{i}", tag=f"wuv{i}") for i in range(2)]
        w_vq = stgA.tile([64, D], F32, name="wvq", tag="wvq")
        w_vk = stgA.tile([64, D], F32, name="wvk", tag="wvk")
        w_vv = stgA.tile([64, D], F32, name="wvv", tag="wvv")
        w_uo = [const.tile([128, R], F32, name=f"wuo{i}", tag=f"wuo{i}") for i in range(2)]
        w_mr = const.tile([64, D], F32, name="wmr", tag="wmr")
        ones = const.tile([128, 1], F32, name="ones", tag="ones")
        czero = const.tile([128, 1], F32, name="czero", tag="czero")
        ceps = const.tile([128, 1], F32, name="ceps", tag="ceps")
        nc.vector.memset(ones[:], 1.0)
        nc.vector.memset(czero[:], 0.0)
        nc.vector.memset(ceps[:], 1e-5)
        nc.const_aps.aps[(F32, 0.0)] = czero
        nc.const_aps.aps[(F32, 1e-5)] = ceps
        vb = {}
        for bi, nm in enumerate(("bnp", "betaf", "gamma", "betaBN")):
            vb[nm] = [const.tile([128, 1], F32, name=f"{nm}{i}", tag=f"{nm}{i}") for i in range(2)]
            for i in range(2):
                nc.sync.dma_start(
                    vb[nm][i][:],
                    wg[OFF_BIAS // 1024:OFF_BIAS // 1024 + 1,
                       256 * bi + 128 * i:256 * bi + 128 * (i + 1)].rearrange(
                           "a (b c) -> (a b) c", c=1))

        nc.sync.dma_start(yq[:], io["yT"][:, :])
        nc.sync.dma_start(yf[:, 0:NHALF], yg[0:64, :])
        nc.sync.dma_start(yf[:, NHALF:N], yg[64:128, :])
        for i in range(2):
            nc.sync.dma_start(w_uq[i][:], wsl(OFF_UQ + 8192 * i, 8192, R))
            nc.sync.dma_start(w_uk[i][:], wsl(OFF_UK + 8192 * i, 8192, R))
            nc.sync.dma_start(w_uv[i][:], wsl(OFF_UV + 8192 * i, 8192, R))
            nc.sync.dma_start(w_uo[i][:], wsl(OFF_UO + 8192 * i, 8192, R))
        nc.sync.dma_start(w_vnp[:], wsl(OFF_VNP, 16384, D))
        nc.sync.dma_start(w_vq[:], wsl(OFF_VQ, 16384, D))
        nc.sync.dma_start(w_vk[:], wsl(OFF_VK, 16384, D))
        nc.sync.dma_start(w_vv[:], wsl(OFF_VV, 16384, D))
        nc.sync.dma_start(w_mr[:], wsl(OFF_MR, 16384, D))

        dosplit(*yf_s, yf[:], tmpS)
        dosplit(*yq_s, yq[:], tmpS)
        dosplit(*w_vnp_s, w_vnp[:], tmpS)
        for i in range(2):
            dosplit(*w_uq_s[i], w_uq[i][:], tmpS)
            dosplit(*w_uk_s[i], w_uk[i][:], tmpS)
        dosplit(*w_vq_s, w_vq[:], tmpS)
        dosplit(*w_vk_s, w_vk[:], tmpS)

        # (lhs hi/lo, rhs hi/lo) 4-pass fp16 split, smallest terms first so
        # the PSUM accumulation rounds ascending-magnitude partial sums
        P3 = ((0, 0), (0, 1), (1, 0), (1, 1))
        NP3 = len(P3) - 1

        # ---------------- stage A: projections (all transposed) -------------
        with tc.tile_pool(name="pjps", bufs=1, space="PSUM") as pjps:
            # hT = Vnp^T @ yT + bnp (keys+vals, full batch); hq on own half
            for (dst, src_s, width, btag) in ((hT, yf_s, N, "pj"),
                                              (hq, yq_s, NHALF, "pjh")):
                for mt in range(2):
                    ps = pjps.tile([128, width], F32, name=btag, tag=btag)
                    for fc in range(width // 512):
                        for pi, (li, ri) in enumerate(P3):
                            nc.tensor.matmul(
                                ps[:, fc * 512:(fc + 1) * 512],
                                lhsT=w_vnp_s[li][:, mt * 128:(mt + 1) * 128],
                                rhs=src_s[ri][:, fc * 512:(fc + 1) * 512],
                                start=(pi == 0), stop=(pi == NP3))
                    nc.vector.tensor_scalar(dst[mt][:], ps[:], ISC2,
                                            vb["bnp"][mt][:], op0=OP.mult,
                                            op1=OP.add)
                    dosplit(*(hT_s[mt] if dst is hT else hq_s[mt]),
                            dst[mt][:], tmpS)
            # aK = Uk^T @ hT; aQ = Uq^T @ hq  (3-pass); aV = Uv^T @ hT (plain)
            for (w_s, h_s, a_sb, a_s, width, btag) in (
                    (w_uk_s, hT_s, aK, aK_s, N, "pj"),
                    (w_uq_s, hq_s, aQ, aQ_s, NHALF, "pjh")):
                ps = pjps.tile([64, width], F32, name=btag, tag=btag)
                for fc in range(width // 512):
                    first, last = True, None
                    for kt in range(2):
                        for pi, (li, ri) in enumerate(P3):
                            nc.tensor.matmul(
                                ps[:, fc * 512:(fc + 1) * 512],
                                lhsT=w_s[kt][li][:],
                                rhs=h_s[kt][ri][:, fc * 512:(fc + 1) * 512],
                                start=(kt == 0 and pi == 0),
                                stop=(kt == 1 and pi == NP3))
                nc.scalar.activation(a_sb[:], ps[:], ACT.Copy, scale=ISC2)
                dosplit(*a_s, a_sb[:], tmpS)
            ps = pjps.tile([64, N], F32, name="pj", tag="pj")
            for kt in range(2):
                for fc in range(4):
                    nc.tensor.matmul(
                        ps[:, fc * 512:(fc + 1) * 512],
                        lhsT=w_uv[kt][:],
                        rhs=hT[kt][:, fc * 512:(fc + 1) * 512],
                        start=(kt == 0), stop=(kt == 1))
            nc.scalar.activation(aV[:], ps[:], ACT.Copy)
            # kT = Vk^T @ aK; qT = Vq^T @ aQ (3-pass, split PSUM direct)
            for (w_s, a_s, dst_s, width, btag) in (
                    (w_vk_s, aK_s, kT_s, N, "pj"),
                    (w_vq_s, aQ_s, qT_s, NHALF, "pjh")):
                for mt in range(2):
                    ps = pjps.tile([128, width], F32, name=btag, tag=btag)
                    for fc in range(width // 512):
                        for pi, (li, ri) in enumerate(P3):
                            nc.tensor.matmul(
                                ps[:, fc * 512:(fc + 1) * 512],
                                lhsT=w_s[li][:, mt * 128:(mt + 1) * 128],
                                rhs=a_s[ri][:, fc * 512:(fc + 1) * 512],
                                start=(pi == 0), stop=(pi == NP3))
                    for j in range(2):
                        dosplit(*dst_s[mt * 2 + j],
                                ps[64 * j:64 * (j + 1), :], tmpS,
                                scale=SC * ISC2)
            # v row-major bf16: per n-tile [128, 256]
            for nt in range(16):
                ps = pjps.tile([128, D], F32, name="pjv", tag="pjv")
                nc.tensor.matmul(
                    ps[:],
                    lhsT=aV[:, nt * 128:(nt + 1) * 128],
                    rhs=w_vv[:], start=True, stop=True)
                nc.scalar.activation(vv[:, nt * D:(nt + 1) * D], ps[:],
                                     ACT.Copy)

        stgA_cm.__exit__(None, None, None)

        # ---------------- stage B: attention --------------------------------
        with tc.tile_pool(name="scps", bufs=1, space="PSUM") as scps, \
             tc.tile_pool(name="ops", bufs=2, space="PSUM") as ops, \
             tc.tile_pool(name="att", bufs=2) as att, \
             tc.tile_pool(name="sml", bufs=3) as sml:
            for h8 in range(H):
                ro = 32 * (h8 % 2)
                qh_ = qT_s[h8 // 2][0][ro:ro + 32, :]
                ql_ = qT_s[h8 // 2][1][ro:ro + 32, :]
                kh_ = kT_s[h8 // 2][0][ro:ro + 32, :]
                kl_ = kT_s[h8 // 2][1][ro:ro + 32, :]
                for nb in range(NBLK):
                    s_ps = scps.tile([128, N], F32, name="s", tag="s")
                    qs = (qh_, ql_)
                    ks = (kh_, kl_)
                    for fc in range(4):
                        for pi, (li, ri) in enumerate(P3):
                            nc.tensor.matmul(
                                s_ps[:, fc * 512:(fc + 1) * 512],
                                lhsT=qs[li][:, nb * 128:(nb + 1) * 128],
                                rhs=ks[ri][:, fc * 512:(fc + 1) * 512],
                                start=(pi == 0), stop=(pi == NP3))
                    e_sb = att.tile([128, N], F32, name="e", tag="e")
                    nc.scalar.activation(e_sb[:], s_ps[:], ACT.Exp,
                                         scale=C_SCALE * ISC2)
                    s_sb = att.tile([128, N], F32, name="sc", tag="sc")
                    nc.scalar.activation(s_sb[:], s_ps[:], ACT.Copy)
                    work = att.tile([128, N], F32, name="wk", tag="wk")
                    nc.scalar.activation(work[:], s_ps[:], ACT.Copy)
                    # exact top-32 on raw scores: 4 rounds of top-8 extract
                    tops = sml.tile([128, 32], F32, name="tops", tag="tops")
                    nc.vector.max(tops[:, 0:8], work[:])
                    for r in range(1, 4):
                        nc.vector.match_replace(work[:],
                                                tops[:, 8 * r - 8:8 * r],
                                                work[:], -3.0e38)
                        nc.vector.max(tops[:, 8 * r:8 * r + 8], work[:])
                    # mask on pristine scores; denominator = sum of kept e
                    attn_f = att.tile([128, N], F32, name="af", tag="af")
                    nc.vector.scalar_tensor_tensor(
                        out=attn_f[:], in0=s_sb[:], scalar=tops[:, 31:32],
                        in1=e_sb[:], op0=OP.is_ge, op1=OP.mult)
                    dn = sml.tile([128, 1], F32, name="dn", tag="dn")
                    nc.vector.reduce_sum(dn[:], attn_f[:], axis=AX)
                    rec = sml.tile([128, 1], F32, name="rec", tag="rec")
                    nc.vector.reciprocal(rec[:], dn[:])
                    attn_b = att.tile([128, N], F16, name="ab", tag="ab")
                    nc.scalar.activation(attn_b[:], attn_f[:], ACT.Copy,
                                         scale=rec[:])
                    eT = att.tile([128, 16, 128], F16, name="eT", tag="eT")
                    for qh in range(4):
                        nc.sync.dma_start_transpose(
                            out=eT[:, 4 * qh:4 * qh + 4, :],
                            in_=attn_b[:, 512 * qh:512 * (qh + 1)].rearrange(
                                "m (di do) -> m di do", do=128))
                    o_ps = ops.tile([32, 128], F32, name="o", tag="o")
                    for mt in range(16):
                        nc.tensor.matmul(
                            o_ps[:],
                            lhsT=vv[:, mt * D + 32 * h8: mt * D + 32 * h8 + 32],
                            rhs=eT[:, mt, :],
                            start=(mt == 0), stop=(mt == 15))
                    nc.scalar.activation(
                        OT[h8 // 4][32 * (h8 % 4):32 * (h8 % 4) + 32,
                                    nb * 128:(nb + 1) * 128], o_ps[:],
                        ACT.Copy)

        # ---------------- stage C: z = Uo^T o, BN coeffs from z-moments -----
        with tc.tile_pool(name="bps", bufs=1, space="PSUM") as bps, \
             tc.tile_pool(name="bsb", bufs=1) as bsb:
            zN = [bsb.tile([128, R], F32, name=f"zN{i}", tag=f"zN{i}")
                  for i in range(NBLK)]
            for nb in range(NBLK):
                ps = bps.tile([128, R], F32, name="zp", tag="zp")
                for kt in range(2):
                    nc.tensor.matmul(
                        ps[:],
                        lhsT=OT[kt][:, nb * 128:(nb + 1) * 128],
                        rhs=w_uo[kt][:],
                        start=(kt == 0), stop=(kt == 1))
                nc.scalar.activation(zN[nb][:], ps[:], ACT.Copy)
                z16 = bsb.tile([128, R], F16, name=f"z16_{nb}", tag=f"z16_{nb}")
                nc.scalar.activation(z16[:], ps[:], ACT.Copy)
                nc.sync.dma_start(outZ[nb * 128:(nb + 1) * 128, :], z16[:])
            # Z2 = z^T z  (64x64, symmetric);  zbar = z^T 1
            z2ps = bps.tile([64, R], F32, name="z2", tag="z2")
            for nb in range(NBLK):
                nc.tensor.matmul(z2ps[:], lhsT=zN[nb][:], rhs=zN[nb][:],
                                 start=(nb == 0), stop=(nb == NBLK - 1))
            zbps = bps.tile([64, 1], F32, name="zb", tag="zb")
            for nb in range(NBLK):
                nc.tensor.matmul(zbps[:], lhsT=zN[nb][:], rhs=ones[:],
                                 start=(nb == 0), stop=(nb == NBLK - 1))
            stat = bsb.tile([64, R + 1], F32, name="stat", tag="stat")
            nc.scalar.activation(stat[:, 0:R], z2ps[:], ACT.Copy)
            nc.scalar.activation(stat[:, R:R + 1], zbps[:], ACT.Copy)
            cc_in = dpool.tile([64, R + 1], F32, name="cc_in")
            cc_out = dpool.tile([64, R + 1], F32, name="cc_out")
            nc.sync.dma_start(cc_in[:], stat[:])
            nc.gpsimd.collective_compute(
                "AllReduce", OP.add,
                replica_groups=[list(range(NCORES))],
                ins=[cc_in.opt()], outs=[cc_out.opt()])
            gst = bsb.tile([64, R + 1], F32, name="gst", tag="gst")
            nc.sync.dma_start(gst[:], cc_out[:])
            # T1 = Z2 @ Mr (Z2 symmetric so lhsT=Z2 works); quad/zproj via ones
            t1ps = bps.tile([64, D], F32, name="t1", tag="t1")
            nc.tensor.matmul(t1ps[:], lhsT=gst[:, 0:R], rhs=w_mr[:],
                             start=True, stop=True)
            mrt1 = bsb.tile([64, D], F32, name="mrt1", tag="mrt1")
            nc.vector.tensor_mul(mrt1[:], w_mr[:], t1ps[:])
            inv_n = 1.0 / float(B * N)
            for mt in range(2):
                qps = bps.tile([128, 1], F32, name=f"q{mt}", tag=f"q{mt}")
                nc.tensor.matmul(qps[:],
                                 lhsT=mrt1[:, mt * 128:(mt + 1) * 128],
                                 rhs=ones[0:64, :], start=True, stop=True)
                pps = bps.tile([128, 1], F32, name=f"p{mt}", tag=f"p{mt}")
                nc.tensor.matmul(pps[:],
                                 lhsT=w_mr[:, mt * 128:(mt + 1) * 128],
                                 rhs=gst[:, R:R + 1], start=True, stop=True)
                betaf = vb["betaf"][mt]
                s1 = bsb.tile([128, 1], F32, name=f"s1{mt}", tag=f"s1{mt}")
                nc.vector.tensor_scalar(s1[:], pps[:], inv_n, None,
                                        op0=OP.mult)
                mu = bsb.tile([128, 1], F32, name=f"mu{mt}", tag=f"mu{mt}")
                nc.vector.tensor_scalar(mu[:], s1[:], betaf[:], None,
                                        op0=OP.add)
                ex2 = bsb.tile([128, 1], F32, name=f"ex{mt}", tag=f"ex{mt}")
                nc.vector.tensor_scalar(ex2[:], qps[:], inv_n, None,
                                        op0=OP.mult)
                cross = bsb.tile([128, 1], F32, name=f"cr{mt}", tag=f"cr{mt}")
                nc.vector.tensor_scalar(cross[:], s1[:], betaf[:], 2.0,
                                        op0=OP.mult, op1=OP.mult)
                bsq = bsb.tile([128, 1], F32, name=f"bq{mt}", tag=f"bq{mt}")
                nc.vector.tensor_mul(bsq[:], betaf[:], betaf[:])
                nc.vector.tensor_add(ex2[:], ex2[:], cross[:])
                nc.vector.tensor_add(ex2[:], ex2[:], bsq[:])
                m2 = bsb.tile([128, 1], F32, name=f"m2{mt}", tag=f"m2{mt}")
                nc.vector.tensor_mul(m2[:], mu[:], mu[:])
                var = bsb.tile([128, 1], F32, name=f"va{mt}", tag=f"va{mt}")
                nc.vector.tensor_sub(var[:], ex2[:], m2[:])
                sd = bsb.tile([128, 1], F32, name=f"sd{mt}", tag=f"sd{mt}")
                nc.scalar.activation(sd[:], var[:], ACT.Sqrt, bias=1e-5)
                rsd = bsb.tile([128, 1], F32, name=f"rs{mt}", tag=f"rs{mt}")
                nc.vector.reciprocal(rsd[:], sd[:])
                a_ch = bsb.tile([128, 1], F32, name=f"ac{mt}", tag=f"ac{mt}")
                nc.vector.tensor_scalar(a_ch[:], vb["gamma"][mt][:], rsd[:],
                                        None, op0=OP.mult)
                nmean = bsb.tile([128, 1], F32, name=f"nm{mt}", tag=f"nm{mt}")
                nc.vector.tensor_scalar(nmean[:], mu[:], a_ch[:], None,
                                        op0=OP.mult)
                bsh = bsb.tile([128, 1], F32, name=f"bs{mt}", tag=f"bs{mt}")
                nc.vector.tensor_sub(bsh[:], vb["betaBN"][mt][:], nmean[:])
                nc.sync.dma_start(
                    outC[0:1, mt * 128:(mt + 1) * 128].rearrange(
                        "a (b c) -> (a b) c", c=1), a_ch[:])
                nc.sync.dma_start(
                    outC[1:2, mt * 128:(mt + 1) * 128].rearrange(
                        "a (b c) -> (a b) c", c=1), bsh[:])

    nc.compile()
    return nc


def _build_runner():
    """Build the Bass program once and wrap it in a single cached jax.jit
    callable (replicates bass2jax.run_bass_via_pjrt's multi-core path, but
    hoisted so warm calls skip retrace/recompile/NEFF-reload)."""
    import jax
    from jax.sharding import Mesh, PartitionSpec
    from jax.experimental.shard_map import shard_map
    from concourse import bass2jax
    from concourse.bass2jax import _bass_exec_p, partition_id_tensor

    nc = _build_program()
    bass2jax.install_neuronx_cc_hook()

    partition_name = (nc.partition_id_tensor.name
                      if nc.partition_id_tensor else None)
    in_names, out_names, out_avals, zero_shapes = [], [], [], []
    for alloc in nc.m.functions[0].allocations:
        if not isinstance(alloc, mybir.MemoryLocationSet):
            continue
        name = alloc.memorylocations[0].name
        if alloc.kind == "ExternalInput":
            if name != partition_name:
                in_names.append(name)
        elif alloc.kind == "ExternalOutput":
            shape = tuple(alloc.tensor_shape)
            dtype = mybir.dt.np(alloc.dtype)
            out_names.append(name)
            out_avals.append(jax.core.ShapedArray(shape, dtype))
            zero_shapes.append((shape, dtype))
    n_params = len(in_names)
    n_outs = len(out_avals)
    all_names = list(in_names) + list(out_names)
    if partition_name is not None:
        all_names.append(partition_name)
    donate = tuple(range(n_params, n_params + n_outs))

    def _body(*args):
        operands = list(args)
        if partition_name is not None:
            operands.append(partition_id_tensor())
        outs = _bass_exec_p.bind(
            *operands,
            out_avals=tuple(out_avals),
            in_names=tuple(all_names),
            out_names=tuple(out_names),
            lowering_input_output_aliases=(),
            sim_require_finite=True,
            sim_require_nnan=True,
            nc=nc,
        )
        return tuple(outs)

    devices = jax.devices()[:NCORES]
    mesh = Mesh(np.asarray(devices), ("core",))
    in_specs = (PartitionSpec("core"),) * (n_params + n_outs)
    out_specs = (PartitionSpec("core"),) * n_outs
    sharded = jax.jit(
        shard_map(_body, mesh=mesh, in_specs=in_specs, out_specs=out_specs,
                  check_rep=False),
        donate_argnums=donate, keep_unused=True)
    return {"fn": sharded, "in_names": in_names, "out_names": out_names,
            "zero_shapes": zero_shapes, "n_params": n_params}


def kernel(**inputs):
    inputs = {k: np.ascontiguousarray(np.asarray(v, np.float32))
              for k, v in inputs.items()}
    if "runner" not in _cached:
        _cached["runner"] = _build_runner()
    rn = _cached["runner"]

    Mr = (inputs["V_o"] @ inputs["U_op"]) @ inputs["V_op"]        # [64, D]
    betaf = inputs["b_o"] @ inputs["U_op"] @ inputs["V_op"] + inputs["b_op"]

    wall = np.zeros((NCORES, WTOT), np.float32)
    pk = np.concatenate([
        inputs["V_np"].ravel(),
        inputs["U_q"].ravel(), inputs["U_k"].ravel(), inputs["U_v"].ravel(),
        inputs["V_q"].ravel(), inputs["V_k"].ravel(), inputs["V_v"].ravel(),
        inputs["U_o"].ravel(), Mr.ravel(),
        inputs["b_np"], betaf, inputs["gamma"], inputs["beta"],
    ]).astype(np.float32)
    wall[0, :pk.size] = pk
    wall = wall.reshape(NCORES * WROWS, 1024)

    x2 = inputs["x"].reshape(B * N, D)
    y = x2 @ inputs["U_np"]                                       # [B*N, 64]
    yT = np.empty((NCORES * R, NHALF), np.float32)
    for c in range(NCORES):
        yT[c * R:(c + 1) * R, :] = y[c * NHALF:(c + 1) * NHALF, :].T

    concats = {"yT": yT, "Wall": wall}
    concat_in = [concats[nm] for nm in rn["in_names"]]
    concat_zeros = [
        np.zeros((NCORES * s[0], *s[1:]), dt) for (s, dt) in rn["zero_shapes"]
    ]
    out_arrs = rn["fn"](*concat_in, *concat_zeros)
    oz = np.asarray(out_arrs[rn["out_names"].index("outZ")])
    oc = np.asarray(out_arrs[rn["out_names"].index("outC")])
    a_ch = oc[0]                                                  # [256]
    bsh = oc[1]
    A = (Mr * a_ch[None, :]).astype(np.float32)                   # [64, D]
    c0 = (betaf * a_ch + bsh).astype(np.float32)
    z = oz.astype(np.float32)                                     # [B*N, 64]
    out = z @ A + c0
    return out.reshape(B, N, D)
